# revision 1
# baseline (speedup 1.0000x reference)
"""Chamfer distance loss kernel for Trainium2 (8 NeuronCores).

Problem: template [4, 8192, 3] f32, source [4, 8192, 3] f32 ->
scalar 0.5*(mean_n sqrt(min_m d2) + mean_m sqrt(min_n d2)) over all batches,
d2 = squared euclidean distance, clamped at 0.

Sharding: core c handles batch b = c//2, template half h = c%2
(4096 template rows x all 8192 source points).

Device algorithm (per core):
  e[n, m] = t.s - 0.5||t||^2 - 0.5||s||^2  = -0.5*d2[n, m]
  computed as a K=13 fp16 split-precision matmul (hi/lo decomposition of
  the coordinates and norms, products accumulated in fp32 PSUM) -- full
  fp32-grade accuracy at 1 cycle/row on the PE.
  Row mins:  d2rowmin = max(-2 * max_m e, 0) -- DVE pairwise fold tree
  (tensor_tensor max at 2x mode) + one small 1x tensor_reduce (batched 8 tiles per reduce).
  Col mins:  partial max_n e accumulated elementwise (DVE tensor_tensor max),
  final partition/core reduction + sqrt/mean on host (tiny arrays).
  Measured: ~317 us HW exec, rel err ~8e-5 (fp16 quantization of e).
"""

import numpy as np

F16 = np.float16
F32 = np.float32

B, N, M, D = 4, 8192, 8192, 3
N_CORES = 8
NSHARD = N // 2          # template rows per core (4096)
NT = NSHARD // 128       # n-tiles per core (32)
MG = M // 2048           # psum groups per n-tile (4)
K = 13                   # augmented contraction dim

_NC_CACHE = {}


def _build_nc():
    import concourse.bacc as bacc
    import concourse.mybir as mybir
    from concourse.tile import TileContext

    f16 = mybir.dt.float16
    f32 = mybir.dt.float32
    Alu = mybir.AluOpType

    nc = bacc.Bacc()
    lhsT = nc.declare_dram_parameter("lhsT", [K, NSHARD], f16, isOutput=False)
    rhs = nc.declare_dram_parameter("rhs", [K, M], f16, isOutput=False)
    rowmax_o = nc.declare_dram_parameter("rowmax", [128, NT], f32, isOutput=True)
    colmax_a_o = nc.declare_dram_parameter("colmaxA", [128, M], f16, isOutput=True)
    colmax_b_o = nc.declare_dram_parameter("colmaxB", [128, M], f16, isOutput=True)

    with TileContext(nc) as tc:
        with (
            tc.tile_pool(name="const", bufs=1) as cpool,
            tc.tile_pool(name="psum", bufs=2, space="PSUM") as ppool,
            tc.tile_pool(name="ebuf", bufs=4) as epool,
        ):
            lhsT_sb = cpool.tile([K, NSHARD], f16)
            nc.gpsimd.dma_start(lhsT_sb[:], lhsT[:])
            # one sbuf tile per m-quarter so the first matmul group only
            # depends on the first quarter's DMA
            rhs_q = []
            for q in range(MG):
                t = cpool.tile([K, M // MG], f16, tag=f"rhsq{q}")
                nc.gpsimd.dma_start(
                    t[:], rhs[:, q * (M // MG):(q + 1) * (M // MG)])
                rhs_q.append(t)

            cmaxA = cpool.tile([128, M], f16)
            cmaxB = cpool.tile([128, M], f16)
            rowmax = cpool.tile([128, NT], f32)
            scratch = cpool.tile([128, M], f16)
            pending = cpool.tile([128, 8 * 512], f16)

            for ti in range(NT):
                e = epool.tile([128, M], f16, tag="e")
                lw = lhsT_sb[:, ti * 128:(ti + 1) * 128]
                for g in range(MG):
                    ps = ppool.tile([128, 2048], f32, tag="ps")
                    for j in range(4):
                        nc.tensor.matmul(
                            ps[:, j * 512:(j + 1) * 512],
                            lw,
                            rhs_q[g][:, j * 512:(j + 1) * 512],
                            start=True,
                            stop=True,
                        )
                    nc.scalar.copy(e[:, g * 2048:(g + 1) * 2048], ps[:])
                # row maxes of this n-tile: pairwise fold tree at 2x, then one
                # 1x max-reduce of the 512-wide remainder
                nc.vector.tensor_tensor(
                    scratch[:, 0:4096], e[:, 0:4096], e[:, 4096:8192], Alu.max)
                nc.vector.tensor_tensor(
                    scratch[:, 4096:6144], scratch[:, 0:2048],
                    scratch[:, 2048:4096], Alu.max)
                nc.vector.tensor_tensor(
                    scratch[:, 6144:7168], scratch[:, 4096:5120],
                    scratch[:, 5120:6144], Alu.max)
                nc.vector.tensor_tensor(
                    scratch[:, 7168:7680], scratch[:, 6144:6656],
                    scratch[:, 6656:7168], Alu.max)
                nc.vector.tensor_tensor(
                    scratch[:, 7680:7936], scratch[:, 7168:7424],
                    scratch[:, 7424:7680], Alu.max)
                blk = ti % 8
                nc.vector.tensor_tensor(
                    pending[:, blk * 128:(blk + 1) * 128], scratch[:, 7680:7808],
                    scratch[:, 7808:7936], Alu.max)
                if blk == 7:
                    # one batched max-reduce for the last 8 tiles' 128-wide folds
                    nc.vector.tensor_reduce(
                        rowmax[:, ti - 7:ti + 1],
                        pending[:, :1024].rearrange("p (b f) -> p b f", f=128),
                        axis=mybir.AxisListType.X, op=Alu.max)
                # col maxes accumulated across n-tiles (2x mode); the first
                # tile of each half is a plain copy (4x mode, no init needed)
                cm = cmaxA if ti < NT // 2 else cmaxB
                if ti % (NT // 2) == 0:
                    # chunked 4x copies so DVE starts right after each convert
                    for g in range(MG):
                        nc.vector.tensor_copy(
                            cm[:, g * 2048:(g + 1) * 2048],
                            e[:, g * 2048:(g + 1) * 2048])
                elif ti == NT - 1:
                    # split the last accumulate by m-halves so the output DMA
                    # overlaps the second half's compute
                    nc.vector.tensor_tensor(
                        cm[:, :M // 2], cm[:, :M // 2], e[:, :M // 2], Alu.max)
                    nc.gpsimd.dma_start(
                        colmax_b_o[:, :M // 2], cm[:, :M // 2])
                    nc.vector.tensor_tensor(
                        cm[:, M // 2:], cm[:, M // 2:], e[:, M // 2:], Alu.max)
                else:
                    nc.vector.tensor_tensor(cm[:], cm[:], e[:], Alu.max)
                if ti == NT // 2 - 1:
                    # first-half col partials ship while the second half computes
                    nc.gpsimd.dma_start(colmax_a_o[:], cmaxA[:])

            nc.gpsimd.dma_start(rowmax_o[:], rowmax[:])
            nc.gpsimd.dma_start(colmax_b_o[:, M // 2:], cmaxB[:, M // 2:])
    return nc


def get_nc():
    if "nc" not in _NC_CACHE:
        nc = _build_nc()
        nc.finalize()
        _NC_CACHE["nc"] = nc
    return _NC_CACHE["nc"]


def _split16(x32):
    """Split fp32 array into (hi, lo) fp16 pair with hi + lo ~= x."""
    hi = x32.astype(F16)
    lo = (x32 - hi.astype(F32)).astype(F16)
    return hi, lo


def _build_lhsT(t):
    """t: [n, 3] f32 template shard -> [13, n] f16 stationary operand."""
    n = t.shape[0]
    th, tl = _split16(t)
    t2 = (t * t).sum(axis=1, dtype=F32)
    u = -0.5 * t2
    uh, ul = _split16(u)
    out = np.empty((K, n), dtype=F16)
    out[0:3] = th.T
    out[3:6] = tl.T
    out[6:9] = th.T
    out[9] = uh
    out[10] = ul
    out[11] = 1.0
    out[12] = 1.0
    return out


def _build_rhs(s):
    """s: [m, 3] f32 source -> [13, m] f16 moving operand."""
    m = s.shape[0]
    sh, sl = _split16(s)
    s2 = (s * s).sum(axis=1, dtype=F32)
    v = -0.5 * s2
    vh, vl = _split16(v)
    out = np.empty((K, m), dtype=F16)
    out[0:3] = sh.T
    out[3:6] = sh.T
    out[6:9] = sl.T
    out[9] = 1.0
    out[10] = 1.0
    out[11] = vh
    out[12] = vl
    return out


def make_in_maps(template, source):
    template = np.asarray(template, dtype=F32)
    source = np.asarray(source, dtype=F32)
    in_maps = []
    for c in range(N_CORES):
        b, h = divmod(c, 2)
        t = template[b, h * NSHARD:(h + 1) * NSHARD]
        s = source[b]
        in_maps.append({"lhsT": _build_lhsT(t), "rhs": _build_rhs(s)})
    return in_maps


def finalize(results):
    """results: list of 8 dicts with 'rowmax' [128, NT] f32, 'colmax' [128, M] f16."""
    row_sqrts = []
    for c in range(N_CORES):
        rm = np.asarray(results[c]["rowmax"], dtype=F32)
        row_sqrts.append(np.sqrt(np.maximum(-2.0 * rm, 0.0), dtype=F32).ravel())
    c01 = np.mean(np.concatenate(row_sqrts), dtype=F32)

    col_sqrts = []
    for b in range(B):
        cm = np.maximum(
            np.maximum(np.asarray(results[2 * b]["colmaxA"]),
                       np.asarray(results[2 * b]["colmaxB"])),
            np.maximum(np.asarray(results[2 * b + 1]["colmaxA"]),
                       np.asarray(results[2 * b + 1]["colmaxB"])),
        ).max(axis=0).astype(F32)  # [M]
        col_sqrts.append(np.sqrt(np.maximum(-2.0 * cm, 0.0), dtype=F32))
    c10 = np.mean(np.concatenate(col_sqrts), dtype=F32)
    return np.float32((c01 + c10) * 0.5)


def kernel(template, source):
    from concourse.bass_utils import run_bass_kernel_spmd

    nc = get_nc()
    in_maps = make_in_maps(template, source)
    res = run_bass_kernel_spmd(nc, in_maps, list(range(N_CORES))).results
    return finalize(res)



# revision 4
# speedup vs baseline: 3.9704x; 3.9704x over previous
"""Chamfer distance loss kernel for Trainium2 (8 NeuronCores) — pruned exact NN.

Problem: template [4, 8192, 3] f32, source [4, 8192, 3] f32 ->
scalar 0.5*(mean_n sqrt(min_m d2) + mean_m sqrt(min_n d2)), d2 clamped at 0.

Algorithm (exact, retrieval-style pruning):
  Host (index-scale work only):
    - For each direction (template->source, source->template) of each batch:
      Morton-sort the queries; per query a certified NN upper bound UB from
      exact distances to 2w Morton-order neighbors in the database ordering.
      Sparse outliers (UB > 2*q90) are re-witnessed wider and, if still
      loose, moved to dedicated outlier tiles.
    - Main tiles = 128 consecutive sorted queries; candidate set = union
      over 16-point sub-tiles of {db points in AABB(sub)+max UB(sub)}.
      Outlier tiles: union of per-point AABB+UB boxes.  Every query's true
      NN is guaranteed inside its tile's candidate set (UBs are real
      distances), so the result is exact.
    - Candidate lists are chunked into 512-wide (small) and 2048-wide (big)
      device jobs, padded with a far dummy point, and bin-packed across the
      8 cores (equal per-core job counts; the compiled program is SPMD).
  Device (all data-scale arithmetic):
    per chunk: stream rhs [13, K] via DMA, one matmul per 512 columns
    computing e = q.s - 0.5||q||^2 - 0.5||db||^2 = -0.5*d2 as a K=13 fp16
    hi/lo split matmul (fp32-grade accuracy), evict PSUM->SBUF fp16 on the
    Act engine, row-max via DVE pairwise fold tree (2x mode), batched
    tensor_reduce every 8 jobs.
  Host finalize: d2 = max(-2*rowmax, 0) scattered by query id (min-combine
  across a query's multiple jobs), sqrt, means.
"""

import numpy as np

F16 = np.float16
F32 = np.float32

B, N, M, D = 4, 8192, 8192, 3
N_CORES = 8
K = 13            # augmented contraction dim
TILE = 128        # queries per job (partition dim)
SUB = 16          # sub-tile granularity for candidate AABBs
KSMALL = 512
KBIG = 2048
DUMMY = 32.0      # far-away padding coordinate

_NC_CACHE = {}


# ----------------------------------------------------------------------------
# host index build
# ----------------------------------------------------------------------------

def _morton_codes(pts, lo, hi, bits=8):
    q = ((pts - lo) / (hi - lo + 1e-9) * (2**bits - 1e-6)).astype(np.uint32)
    code = np.zeros(len(pts), np.uint64)
    for b in range(bits):
        for dd in range(3):
            code |= ((q[:, dd].astype(np.uint64) >> b) & 1) << np.uint64(3 * b + dd)
    return code


def _witness_ub(qs, dbs, pos, w):
    n = len(dbs)
    offs = np.arange(-w, w)
    idx = np.clip(pos[:, None] + offs[None, :], 0, n - 1)
    d2 = ((qs[:, None, :] - dbs[idx]) ** 2).sum(-1)
    return np.sqrt(d2.min(1))


def _build_direction(q_pts, db_pts):
    """Returns list of (qids_orig[128 padded -1], cand_db_orig_indices)."""
    lo = np.minimum(q_pts.min(0), db_pts.min(0))
    hi = np.maximum(q_pts.max(0), db_pts.max(0))
    qc = _morton_codes(q_pts, lo, hi)
    dbc = _morton_codes(db_pts, lo, hi)
    q_order = np.argsort(qc, kind="stable")
    db_order = np.argsort(dbc, kind="stable")
    qs = q_pts[q_order]
    dbs = db_pts[db_order]
    pos = np.searchsorted(dbc[db_order], qc[q_order])

    ub = _witness_ub(qs, dbs, pos, 32)
    thr = np.quantile(ub, 0.90) * 2.0
    out = ub > thr
    if out.any():
        ub[out] = _witness_ub(qs[out], dbs, pos[out], 256)
        out = ub > thr

    tiles = []
    nq = len(q_pts)
    for t in range(nq // TILE):
        sl = slice(t * TILE, (t + 1) * TILE)
        keep = ~out[sl]
        if not keep.any():
            continue
        tp, r = qs[sl], ub[sl]
        mask = np.zeros(len(db_pts), bool)
        for s in range(0, TILE, SUB):
            kk = keep[s:s + SUB]
            if not kk.any():
                continue
            tps = tp[s:s + SUB][kk]
            R = r[s:s + SUB][kk].max() + 1e-4
            mask |= ((db_pts >= tps.min(0) - R) &
                     (db_pts <= tps.max(0) + R)).all(1)
        cand = np.nonzero(mask)[0]
        qids = q_order[t * TILE:(t + 1) * TILE].copy()
        qids[~keep] = -1
        tiles.append((qids, cand))

    oidx = np.nonzero(out)[0]
    for s in range(0, len(oidx), TILE):
        grp = oidx[s:s + TILE]
        tp, r = qs[grp], ub[grp]
        mask = np.zeros(len(db_pts), bool)
        for i in range(len(grp)):
            mask |= ((db_pts >= tp[i] - (r[i] + 1e-4)) &
                     (db_pts <= tp[i] + (r[i] + 1e-4))).all(1)
        cand = np.nonzero(mask)[0]
        qids = np.full(TILE, -1, np.int64)
        qids[:len(grp)] = q_order[grp]
        tiles.append((qids, cand))
    return tiles


def _chunk_tiles(tiles):
    """Split each tile's candidate list into KBIG/KSMALL chunks.
    Returns list of (qids, cand_chunk, cls) with cls in ('s','b')."""
    chunks = []
    for qids, cand in tiles:
        c = len(cand)
        off = 0
        nb = c // KBIG
        rem = c - nb * KBIG
        if rem > 3 * KSMALL:
            nb += 1
            rem = 0
        for _ in range(nb):
            chunks.append((qids, cand[off:off + KBIG], "b"))
            off += KBIG
        while rem > 0:
            take = min(rem, KSMALL)
            chunks.append((qids, cand[off:off + take], "s"))
            off += take
            rem -= take
    return chunks


# ----------------------------------------------------------------------------
# fp16 split operand builders (same math as the dense baseline)
# ----------------------------------------------------------------------------

def _split16(x32):
    hi = x32.astype(F16)
    lo = (x32 - hi.astype(F32)).astype(F16)
    return hi, lo


def _build_lhsT(t):
    """t: [n, 3] f32 queries -> [13, n] f16 stationary operand."""
    n = t.shape[0]
    th, tl = _split16(t)
    u = -0.5 * (t * t).sum(axis=1, dtype=F32)
    uh, ul = _split16(u)
    out = np.empty((K, n), dtype=F16)
    out[0:3] = th.T
    out[3:6] = tl.T
    out[6:9] = th.T
    out[9] = uh
    out[10] = ul
    out[11] = 1.0
    out[12] = 1.0
    return out


def _build_rhs(s):
    """s: [m, 3] f32 db points -> [13, m] f16 moving operand."""
    m = s.shape[0]
    sh, sl = _split16(s)
    v = -0.5 * (s * s).sum(axis=1, dtype=F32)
    vh, vl = _split16(v)
    out = np.empty((K, m), dtype=F16)
    out[0:3] = sh.T
    out[3:6] = sh.T
    out[6:9] = sl.T
    out[9] = 1.0
    out[10] = 1.0
    out[11] = vh
    out[12] = vl
    return out


# ----------------------------------------------------------------------------
# schedule build: global chunk list -> per-core equal-shape job arrays
# ----------------------------------------------------------------------------

def prepare(template, source):
    """Host index build. Returns (order, in_maps, slot_qids) where
    order is the per-core class sequence (shared), in_maps the per-core
    parameter dicts, slot_qids[core][slot] = int32 [128] global output ids."""
    template = np.asarray(template, dtype=F32)
    source = np.asarray(source, dtype=F32)

    chunks = []   # (encoded qids[128], cand_pts [c,3], q_full, qids_orig, cnt)
    for b in range(B):
        for di, (q, db) in enumerate(((template[b], source[b]),
                                      (source[b], template[b]))):
            base = di * (B * N) + b * N
            for qids, cand, _cls in _chunk_tiles(_build_direction(q, db)):
                g = np.where(qids >= 0, qids + base, -1).astype(np.int64)
                chunks.append((g, db[cand], q, qids, len(cand)))

    # split into classes
    small = [c for c in chunks if len(c[1]) <= KSMALL]
    big = [c for c in chunks if len(c[1]) > KSMALL]

    def pad8(lst, kpad):
        while len(lst) % N_CORES:
            lst.append((np.full(TILE, -1, np.int64),
                        np.zeros((0, 3), F32), None, None, 0))
        return lst

    small = pad8(small, KSMALL)
    big = pad8(big, KBIG)
    ns, nb = len(small) // N_CORES, len(big) // N_CORES

    # order: interleave bigs evenly among smalls
    order = []
    if nb:
        step = max(1, (ns + nb) // nb)
    si = bi = 0
    pos = 0
    while si < ns or bi < nb:
        if bi < nb and (si >= ns or (pos % step == step - 1)):
            order.append("b")
            bi += 1
        else:
            order.append("s")
            si += 1
        pos += 1

    # sort chunk lists by candidate count desc, deal round-robin for balance
    small.sort(key=lambda c: -len(c[1]))
    big.sort(key=lambda c: -len(c[1]))
    per_core_small = [small[c::N_CORES] for c in range(N_CORES)]
    per_core_big = [big[c::N_CORES] for c in range(N_CORES)]

    nslots = ns + nb
    in_maps = []
    slot_qids = []
    for c in range(N_CORES):
        lhsT_all = np.zeros((K, nslots * TILE), F16)
        rhs_s = np.empty((K, max(ns, 1) * KSMALL), F16)
        rhs_b = np.empty((K, max(nb, 1) * KBIG), F16)
        qid_arr = np.full((nslots, TILE), -1, np.int64)
        sit = iter(range(ns))
        bit = iter(range(nb))
        s_list, b_list = per_core_small[c], per_core_big[c]
        for slot, cls in enumerate(order):
            if cls == "s":
                i = next(sit)
                g, cand_pts, qpts, qidx, cnt = s_list[i]
                kw = KSMALL
                dst = rhs_s[:, i * KSMALL:(i + 1) * KSMALL]
            else:
                i = next(bit)
                g, cand_pts, qpts, qidx, cnt = b_list[i]
                kw = KBIG
                dst = rhs_b[:, i * KBIG:(i + 1) * KBIG]
            qid_arr[slot] = g
            pts = np.full((kw, 3), DUMMY, F32)
            pts[:cnt] = cand_pts
            dst[:] = _build_rhs(pts)
            if qpts is not None:
                qp = np.zeros((TILE, 3), F32)
                live = qidx >= 0
                qp[live] = qpts[qidx[live]]   # qidx = original indices
                lhsT_all[:, slot * TILE:(slot + 1) * TILE] = _build_lhsT(qp)
        in_maps.append({"lhsT": lhsT_all, "rhs_s": rhs_s, "rhs_b": rhs_b})
        slot_qids.append(qid_arr)
    return tuple(order), in_maps, slot_qids


# ----------------------------------------------------------------------------
# device program
# ----------------------------------------------------------------------------

def _build_nc(order):
    import concourse.bacc as bacc
    import concourse.mybir as mybir
    from concourse.tile import TileContext

    f16 = mybir.dt.float16
    f32 = mybir.dt.float32
    Alu = mybir.AluOpType

    ns = order.count("s")
    nb = order.count("b")
    nslots = len(order)

    nc = bacc.Bacc()
    lhsT = nc.declare_dram_parameter("lhsT", [K, nslots * TILE], f16,
                                     isOutput=False)
    rhs_s = nc.declare_dram_parameter("rhs_s", [K, max(ns, 1) * KSMALL], f16,
                                      isOutput=False)
    rhs_b = nc.declare_dram_parameter("rhs_b", [K, max(nb, 1) * KBIG], f16,
                                      isOutput=False)
    rowmax_o = nc.declare_dram_parameter("rowmax", [128, nslots], f32,
                                         isOutput=True)

    with TileContext(nc) as tc:
        with (
            tc.tile_pool(name="const", bufs=1) as cpool,
            tc.tile_pool(name="rs", bufs=6) as rspool,
            tc.tile_pool(name="rb", bufs=2) as rbpool,
            tc.tile_pool(name="psum", bufs=8, space="PSUM") as ppool,
            tc.tile_pool(name="es", bufs=4) as espool,
            tc.tile_pool(name="eb", bufs=2) as ebpool,
            tc.tile_pool(name="scr", bufs=4) as scpool,
        ):
            lhsT_sb = cpool.tile([K, nslots * TILE], f16)
            nc.gpsimd.dma_start(lhsT_sb[:], lhsT[:])
            pending = cpool.tile([128, 8 * 64], f16)
            rowmax = cpool.tile([128, nslots], f32)

            si = bi = 0
            for slot, cls in enumerate(order):
                lw = lhsT_sb[:, slot * TILE:(slot + 1) * TILE]
                if cls == "s":
                    rt = rspool.tile([K, KSMALL], f16, tag="rs")
                    nc.gpsimd.dma_start(
                        rt[:], rhs_s[:, si * KSMALL:(si + 1) * KSMALL])
                    ps = ppool.tile([128, 512], f32, tag="ps")
                    nc.tensor.matmul(ps[:], lw, rt[:], start=True, stop=True)
                    e = espool.tile([128, KSMALL], f16, tag="es")
                    nc.scalar.copy(e[:], ps[:])
                    scr = scpool.tile([128, 384], f16, tag="scs")
                    nc.vector.tensor_tensor(
                        scr[:, 0:256], e[:, 0:256], e[:, 256:512], Alu.max)
                    nc.vector.tensor_tensor(
                        scr[:, 256:384], scr[:, 0:128], scr[:, 128:256],
                        Alu.max)
                    blk = slot % 8
                    nc.vector.tensor_tensor(
                        pending[:, blk * 64:(blk + 1) * 64],
                        scr[:, 256:320], scr[:, 320:384], Alu.max)
                    si += 1
                else:
                    rt = rbpool.tile([K, KBIG], f16, tag="rb")
                    nc.gpsimd.dma_start(
                        rt[:], rhs_b[:, bi * KBIG:(bi + 1) * KBIG])
                    e = ebpool.tile([128, KBIG], f16, tag="eb")
                    for c4 in range(4):
                        ps = ppool.tile([128, 512], f32, tag="ps")
                        nc.tensor.matmul(
                            ps[:], lw, rt[:, c4 * 512:(c4 + 1) * 512],
                            start=True, stop=True)
                        nc.scalar.copy(e[:, c4 * 512:(c4 + 1) * 512], ps[:])
                    scr = scpool.tile([128, 1984], f16, tag="scb")
                    nc.vector.tensor_tensor(
                        scr[:, 0:1024], e[:, 0:1024], e[:, 1024:2048], Alu.max)
                    nc.vector.tensor_tensor(
                        scr[:, 1024:1536], scr[:, 0:512], scr[:, 512:1024],
                        Alu.max)
                    nc.vector.tensor_tensor(
                        scr[:, 1536:1792], scr[:, 1024:1280],
                        scr[:, 1280:1536], Alu.max)
                    nc.vector.tensor_tensor(
                        scr[:, 1792:1920], scr[:, 1536:1664],
                        scr[:, 1664:1792], Alu.max)
                    blk = slot % 8
                    nc.vector.tensor_tensor(
                        pending[:, blk * 64:(blk + 1) * 64],
                        scr[:, 1792:1856], scr[:, 1856:1920], Alu.max)
                    bi += 1
                if slot % 8 == 7 or slot == nslots - 1:
                    nblk = (slot % 8) + 1
                    nc.vector.tensor_reduce(
                        rowmax[:, slot - nblk + 1:slot + 1],
                        pending[:, :nblk * 64].rearrange(
                            "p (b f) -> p b f", f=64),
                        axis=mybir.AxisListType.X, op=Alu.max)

            nc.gpsimd.dma_start(rowmax_o[:], rowmax[:])
    return nc


def get_nc(order):
    key = tuple(order)
    if key not in _NC_CACHE:
        nc = _build_nc(key)
        nc.finalize()
        _NC_CACHE[key] = nc
    return _NC_CACHE[key]


# ----------------------------------------------------------------------------
# finalize
# ----------------------------------------------------------------------------

def finalize(results, slot_qids):
    d2_all = np.full(2 * B * N, np.inf, dtype=F32)
    for c in range(N_CORES):
        rm = np.asarray(results[c]["rowmax"], dtype=F32)   # [128, nslots]
        qid = slot_qids[c]                                  # [nslots, 128]
        vals = np.maximum(-2.0 * rm.T, 0.0)                 # [nslots, 128] d2
        live = qid >= 0
        np.minimum.at(d2_all, qid[live], vals[live])
    assert np.isfinite(d2_all).all(), "query coverage hole"
    d = np.sqrt(d2_all, dtype=F32)
    c01 = d[:B * N].mean(dtype=F32)
    c10 = d[B * N:].mean(dtype=F32)
    return np.float32((c01 + c10) * 0.5)


def kernel(template, source):
    from concourse.bass_utils import run_bass_kernel_spmd

    order, in_maps, slot_qids = prepare(template, source)
    nc = get_nc(order)
    res = run_bass_kernel_spmd(nc, in_maps, list(range(N_CORES))).results
    return finalize(res, slot_qids)
